# revision 24
# baseline (speedup 1.0000x reference)
"""Trainium2 Bass kernel for nn_MultiHeadAttention_16346645529223.

Full inputs in / full output out. Sharding: (batch x head-half) over the 8
cores -- core c handles batch b = c//2 and heads g*8..g*8+7 with g = c%2.
No device collectives: each core emits the partial output projection
partial_g = heads_g @ wo[g-rows] and the host adds the two partials per
batch (untimed assembly, like the baseline's concat).

Per-core pipeline (b fixed, 8 heads = 4 head-pairs, S = 2048):
  - qT/kT projections with weight-stationary matmuls -> [128 (2 heads x 64
    dims), S] bf16 tiles per pair; softmax scale folded into wq/bq.
  - v projected directly in [kv, dims] orientation (value chunk stationary,
    wv moving) -- no PE transposes. Stored per (pair, kv chunk) as
    [ones 0:64 | A dims | B dims | ones 192:256]: the 64 replicated ones
    columns make each AV matmul write the softmax denominator across 64
    PSUM partitions (A: den 0:64 / dims 64:128; B: dims 0:64 / den 64:128),
    so normalization is a cross-partition-base reciprocal (PSUM->SBUF,
    legal when only one operand is in SBUF) plus an aligned multiply --
    no gpsimd broadcast, no DMA hop, no PSUM-evacuation copies.
  - scores computed transposed S^T[kv, q] with the two heads of a pair in
    the two 64-row halves of the PE array (concurrent row-group matmuls),
    both heads' chunks in one 2-bank PSUM tile.
  - exp via one ScalarE activation per [128, 1024] PSUM tile -> bf16 xs.
  - AV accumulated over 16 kv chunks (M=128 stationaries incl. the ones).
  - q/k/o projections split into ~2-matmul closures and paced into the
    attention stream (urgent kT with per-pair deadlines, lazy qproj/oproj)
    so PE keeps running during ACT-gated stretches and no engine sees a
    long foreign block.
  - output projection from SBUF heads (per pair ordered [odd|even], wo rows
    reordered host-side to match), + (bo + bv @ wo) bias on g==0 cores
    (zeros on g==1), f32 [1024, S] partial out.

A `repeat` knob wraps the whole per-call body in a tc.For_i loop (consts/
weights stay loaded) so test.py can measure pure device time as the slope
(wall(R) - wall(1)) / (R - 1), cancelling the ~70-90 ms axon dispatch
overhead that dominated the old wall-clock measurement.
"""

import sys

sys.path.insert(0, "/opt/trn_rl_repo")

import numpy as np

N_CORES = 8
B, S, D, H, DH = 4, 2048, 1024, 16, 64
HG = H // 2   # heads per core
NP = HG // 2  # head pairs per core
NDC = D // 128

# vh column layout per (pair, kv-chunk): [ones 0:64 | A dims 64:128 |
# B dims 128:192 | ones 192:256].  64 replicated ones columns mean the AV
# matmuls produce the softmax denominator on 64 PSUM partitions directly
# (A: den at parts 0:64, dims at 64:128; B: dims at parts 0:64, den at
# 64:128), so normalization is just reciprocal+mul -- no partition
# broadcast, no DMA hop, no gpsimd.
VHW = 256


def build_kernel(nc, s_sz=S, repeat=1, phase="full", sc_bufs=2, share_work=False):
    import concourse.tile as tile
    import concourse.mybir as mybir
    from contextlib import ExitStack

    f32 = mybir.dt.float32
    bf16 = mybir.dt.bfloat16
    EXP = mybir.ActivationFunctionType.Exp

    S_ = s_sz
    NQT = S_ // 512
    NKV = S_ // 512
    NKC = S_ // 128

    query_t = nc.dram_tensor("query_t", [D, S_], bf16, kind="ExternalInput")
    value_t = nc.dram_tensor("value_t", [D, S_], bf16, kind="ExternalInput")
    wq_d = nc.dram_tensor("wq", [D, 512], bf16, kind="ExternalInput")
    wk_d = nc.dram_tensor("wk", [D, 512], bf16, kind="ExternalInput")
    wv_d = nc.dram_tensor("wv", [D, 512], bf16, kind="ExternalInput")
    wo_d = nc.dram_tensor("wo", [512, D], bf16, kind="ExternalInput")
    bq_d = nc.dram_tensor("bq", [128, NP], f32, kind="ExternalInput")
    bk_d = nc.dram_tensor("bk", [128, NP], f32, kind="ExternalInput")
    bo_d = nc.dram_tensor("bo", [128, 8], f32, kind="ExternalInput")
    out_d = nc.dram_tensor("out_t", [D, S_], f32, kind="ExternalOutput")

    with tile.TileContext(nc) as tc, ExitStack() as ctx:
        consts = ctx.enter_context(tc.tile_pool(name="consts", bufs=1))
        persist = ctx.enter_context(tc.tile_pool(name="persist", bufs=1))
        stage = ctx.enter_context(tc.tile_pool(name="stage", bufs=2))
        xsp = ctx.enter_context(tc.tile_pool(name="xsp", bufs=10))
        small = ctx.enter_context(tc.tile_pool(name="small", bufs=2))
        pp = ctx.enter_context(tc.tile_pool(name="pp", bufs=2, space="PSUM"))
        scp = ctx.enter_context(tc.tile_pool(name="scp", bufs=sc_bufs, space="PSUM"))
        if share_work:
            avp = pp
        else:
            avp = ctx.enter_context(tc.tile_pool(name="avp", bufs=2, space="PSUM"))

        wq_sb = consts.tile([128, NDC, 512], bf16)
        nc.sync.dma_start(out=wq_sb[:], in_=wq_d[:].rearrange("(dc p) m -> p dc m", p=128))
        wk_sb = consts.tile([128, NDC, 512], bf16)
        nc.sync.dma_start(out=wk_sb[:], in_=wk_d[:].rearrange("(dc p) m -> p dc m", p=128))
        wv_sb = consts.tile([128, NDC, 512], bf16)
        nc.sync.dma_start(out=wv_sb[:], in_=wv_d[:].rearrange("(dc p) m -> p dc m", p=128))
        wo_sb = consts.tile([128, NP, D], bf16)
        nc.sync.dma_start(out=wo_sb[:], in_=wo_d[:].rearrange("(dc p) m -> p dc m", p=128))
        bq_sb = consts.tile([128, NP], f32)
        nc.sync.dma_start(out=bq_sb[:], in_=bq_d[:])
        bk_sb = consts.tile([128, NP], f32)
        nc.sync.dma_start(out=bk_sb[:], in_=bk_d[:])
        bo_sb = consts.tile([128, 8], f32)
        nc.sync.dma_start(out=bo_sb[:], in_=bo_d[:])


        qT = persist.tile([128, NP, S_], bf16)
        kT = persist.tile([128, NP, S_], bf16)
        vh = persist.tile([128, NP, NKC, VHW], bf16)
        heads = persist.tile([128, NP, S_], bf16)

        # static parts of vh (replicated ones blocks)
        nc.vector.memset(vh[:, :, :, 0:64], 1.0)
        nc.vector.memset(vh[:, :, :, 192:VHW], 1.0)

        def qproj_closures(qt, tag_sfx=""):
            """Split q projection into small closures for filler pacing."""
            qsl = slice(qt * 512, (qt + 1) * 512)
            state = {}

            def dma():
                qst = stage.tile([128, NDC, 512], bf16, tag="qst",
                                 name=f"qst_{qt}{tag_sfx}")
                nc.sync.dma_start(out=qst[:], in_=query_t[:, qsl].rearrange(
                    "(dc p) m -> p dc m", p=128))
                state["qst"] = qst

            cls = [dma]
            for p in range(NP):
                for dch in range(NDC // 2):
                    def mm2(p=p, dch=dch, first=dch == 0):
                        if first:
                            alloc_p = pp.tile([128, 512], f32, tag="pp",
                                              name=f"psq_{p}_{qt}{tag_sfx}")
                            state["ps"] = alloc_p
                        ps = state["ps"]
                        for dc in (2 * dch, 2 * dch + 1):
                            nc.tensor.matmul(ps[:], wq_sb[:, dc, p * 128:(p + 1) * 128],
                                             state["qst"][:, dc, :],
                                             start=dc == 0, stop=dc == NDC - 1)
                    cls.append(mm2)

                def add(p=p):
                    nc.vector.tensor_scalar_add(qT[:, p, qsl], state["ps"][:],
                                                bq_sb[:, p:p + 1])
                cls.append(add)
            return cls

        def qproj(qt, tag_sfx=""):
            for c in qproj_closures(qt, tag_sfx):
                c()

        def body():
            qproj(0)
            vsts = []
            for kvt in range(NKV):
                ksl = slice(kvt * 512, (kvt + 1) * 512)
                vst = stage.tile([128, NDC, 512], bf16, tag="vst", bufs=NKV,
                                 name=f"vst_{kvt}")
                nc.sync.dma_start(out=vst[:], in_=value_t[:, ksl].rearrange("(dc p) m -> p dc m", p=128))
                vsts.append(vst)

            def kTproj_closures(p):
                cls = []
                state = {}
                for kvt in range(NKV):
                    ksl = slice(kvt * 512, (kvt + 1) * 512)
                    for dch in range(NDC // 2):
                        def mm2(p=p, kvt=kvt, dch=dch, first=dch == 0):
                            if first:
                                state["ps"] = pp.tile([128, 512], f32, tag="pp",
                                                      name=f"psk_{p}_{kvt}")
                            ps = state["ps"]
                            for dc in (2 * dch, 2 * dch + 1):
                                nc.tensor.matmul(ps[:], wk_sb[:, dc, p * 128:(p + 1) * 128],
                                                 vsts[kvt][:, dc, :],
                                                 start=dc == 0, stop=dc == NDC - 1)
                        cls.append(mm2)

                    def add(p=p, kvt=kvt, ksl=ksl):
                        nc.vector.tensor_scalar_add(kT[:, p, ksl], state["ps"][:],
                                                    bk_sb[:, p:p + 1])
                    cls.append(add)
                return cls

            def kTproj(p, kvt):
                cls = kTproj_closures(p)
                for c in cls[kvt * 5:(kvt + 1) * 5]:
                    c()

            def vproj(kc):
                kvt, kcr = divmod(kc, 4)
                ps = pp.tile([128, 512], f32, tag="pp", name=f"psv_{kc}")
                for dc in range(NDC):
                    nc.tensor.matmul(ps[:], vsts[kvt][:, dc, kcr * 128:(kcr + 1) * 128],
                                     wv_sb[:, dc, :], start=dc == 0, stop=dc == NDC - 1)
                pr = ps[:].rearrange("p (g t e) -> p g t e", t=2, e=64)
                nc.vector.tensor_copy(vh[:, :, kc, 64:128], pr[:, :, 0, :])
                nc.vector.tensor_copy(vh[:, :, kc, 128:192], pr[:, :, 1, :])

            # kT for pair 0 up front; everything else (v-proj, kT of pairs
            # 1..3) is interleaved into the first q-tile's attention stream
            # below so ACT starts exping almost immediately.
            for kvt in range(NKV):
                kTproj(0, kvt)

            if phase == "proj":
                nc.sync.dma_start(out=out_d[0:128, 0:S_ // 2], in_=qT[:, 0, :].bitcast(f32))
                nc.sync.dma_start(out=out_d[128:256, 0:S_ // 2], in_=kT[:, 0, :].bitcast(f32))
                nc.sync.dma_start(out=out_d[256:384, 0:VHW * NKC // 2], in_=vh[:, 0, :, :].bitcast(f32))
                return

            def oproj_closures(qt):
                """Per output-chunk: 2 matmul closures + 1 add/dma closure."""
                qsl = slice(qt * 512, (qt + 1) * 512)
                state = {}
                cls = []
                for ec in range(8):
                    for ph in range(2):
                        def mm2(ec=ec, ph=ph):
                            if ph == 0:
                                state["ps"] = pp.tile([128, 512], f32, tag="pp",
                                                      name=f"pso_{qt}_{ec}")
                            ps = state["ps"]
                            for p in (2 * ph, 2 * ph + 1):
                                nc.tensor.matmul(ps[:], wo_sb[:, p, ec * 128:(ec + 1) * 128],
                                                 heads[:, p, qsl],
                                                 start=p == 0, stop=p == NP - 1)
                        cls.append(mm2)

                    def add_dma(ec=ec):
                        osb = small.tile([128, 512], f32, tag="osb", name=f"osb_{qt}_{ec}")
                        nc.vector.tensor_scalar_add(osb[:], state["ps"][:], bo_sb[:, ec:ec + 1])
                        nc.sync.dma_start(out=out_d[ec * 128:(ec + 1) * 128, qsl], in_=osb[:])
                    cls.append(add_dma)
                return cls

            def oproj(qt):
                for c in oproj_closures(qt):
                    c()

            for qt in range(NQT):
                qsl = slice(qt * 512, (qt + 1) * 512)
                # lazy fillers: next q-tile's projection + previous q-tile's
                # output projection, paced one small closure at a time into
                # the attention stream so PE never runs a long foreign block
                # while ACT starves or DVE queues back up.
                lazy = []
                if qt + 1 < NQT:
                    lazy.extend(qproj_closures(qt + 1))
                if phase in ("full", "fullnb") and qt > 0:
                    lazy.extend(oproj_closures(qt - 1))
                lazy_total = len(lazy)
                lazy_emitted = 0
                SLOTS = NP * NKC

                for p in range(NP):
                    do_av = phase not in ("scnx", "scores")
                    if do_av:
                        avA = avp.tile([128, 512], f32, tag="pp" if share_work else "av",
                                       name=f"av_{p}_{qt}_0")
                        avB = avp.tile([128, 512], f32, tag="pp" if share_work else "av",
                                       name=f"av_{p}_{qt}_1")
                    xst = {}

                    def av_mms(kc):
                        # AV matmuls interleaved into the scores stream; the
                        # two heads accumulate in separate banks, scores use
                        # other banks, so the accumulation groups don't mix.
                        nc.tensor.matmul(avA[:], vh[:, p, kc, 0:128],
                                         xst[kc][:, 0:512],
                                         start=kc == 0, stop=kc == NKC - 1)
                        nc.tensor.matmul(avB[:], vh[:, p, kc, 128:VHW],
                                         xst[kc][:, 512:1024],
                                         start=kc == 0, stop=kc == NKC - 1)

                    # kT for the next pair: urgent fillers that must finish
                    # within this pair's stream (scores of pair p+1 need them)
                    urgent = kTproj_closures(p + 1) if (qt == 0 and p + 1 < NP) else []
                    urgent_emitted = 0

                    LAG = 3
                    for kc in range(NKC):
                        if qt == 0 and p == 0:
                            vproj(kc)
                        kcl = slice(kc * 128, (kc + 1) * 128)
                        sc = scp.tile([128, 1024], f32, tag="sc", name=f"sc_{p}_{qt}_{kc}")
                        nc.tensor.matmul(sc[:, 0:512], kT[0:64, p, kcl], qT[0:64, p, qsl],
                                         start=True, stop=True)
                        nc.tensor.matmul(sc[:, 512:1024], kT[64:128, p, kcl], qT[64:128, p, qsl],
                                         start=True, stop=True)
                        if phase != "scnx":
                            x = xsp.tile([128, 1024], bf16, tag="xs", name=f"xs_{p}_{qt}_{kc}")
                            nc.scalar.activation(x[:], sc[:], EXP)
                            xst[kc] = x
                        if do_av and kc >= LAG:
                            av_mms(kc - LAG)
                        # pacing: keep urgent (kT) on deadline, trickle lazy
                        if urgent:
                            utarget = (len(urgent) * (kc + 1) + NKC - 1) // NKC
                            while urgent_emitted < utarget:
                                urgent[urgent_emitted]()
                                urgent_emitted += 1
                        slot = p * NKC + kc
                        if lazy and slot >= 4:
                            ltarget = min(lazy_total,
                                          lazy_total * (slot - 3) // (SLOTS - 12))
                            while lazy_emitted < ltarget:
                                lazy[lazy_emitted]()
                                lazy_emitted += 1
                    while urgent_emitted < len(urgent):
                        urgent[urgent_emitted]()
                        urgent_emitted += 1
                    if not do_av:
                        continue
                    for kc in range(NKC - LAG, NKC):
                        av_mms(kc)
                    if phase == "av":
                        if p == 0 and qt == 0:
                            avdbg = small.tile([128, 512], f32, tag="avdbg", name="avsb_dbg")
                            nc.vector.tensor_copy(avdbg[:], avB[:])
                            nc.sync.dma_start(out=out_d[128:256, 0:512], in_=avdbg[:])
                        continue
                    # normalization: the replicated-ones AV columns already
                    # put the denominator on 64 PSUM partitions, so this is
                    # one cross-base reciprocal (PSUM->SBUF) + one aligned
                    # multiply per head, straight from PSUM.
                    recA = small.tile([128, 512], f32, tag="recA", name=f"recA_{p}_{qt}")
                    nc.vector.reciprocal(recA[64:128, :], avA[0:64, :])
                    nc.vector.tensor_mul(heads[64:128, p, qsl], avA[64:128, :], recA[64:128, :])
                    recB = small.tile([128, 512], f32, tag="recB", name=f"recB_{p}_{qt}")
                    nc.vector.reciprocal(recB[0:64, :], avB[64:128, :])
                    nc.vector.tensor_mul(heads[0:64, p, qsl], avB[0:64, :], recB[0:64, :])
                # drain any lazy fillers not yet emitted (qproj must complete
                # before the next q-tile's scores read qT)
                while lazy_emitted < lazy_total:
                    lazy[lazy_emitted]()
                    lazy_emitted += 1
            if phase in ("full", "fullnb"):
                oproj(NQT - 1)

        if repeat == 1:
            body()
        else:
            # big body (>256 insts/engine) -> arm branch prefetch so the
            # back-edge I$-hits instead of a ~4us IRAM refetch stall
            hints = (mybir.EngineType.PE, mybir.EngineType.Activation,
                     mybir.EngineType.DVE, mybir.EngineType.SP,
                     mybir.EngineType.Pool)
            with tc.For_i(0, repeat, hint_engines=hints):
                body()

    nc.finalize()
    return nc


def make_in_maps(query, value, wq, bq, wk, bk, wv, bv, wo, bo, s_sz=S):
    """Host-side prep: transpose activations, fold scale/bv, shard per core."""
    import ml_dtypes

    bf16 = ml_dtypes.bfloat16
    scale = np.float32(1.0 / np.sqrt(np.float32(DH)))
    b_cnt = query.shape[0]

    q_t = [np.ascontiguousarray(query[b].T).astype(bf16) for b in range(b_cnt)]
    v_t = [np.ascontiguousarray(value[b].T).astype(bf16) for b in range(b_cnt)]

    in_maps = []
    for c in range(N_CORES):
        b, g = c // 2, c % 2
        hs = [g * HG + i for i in range(HG)]
        wq_c = np.concatenate([wq[h] * scale for h in hs], axis=1)  # [D, 512]
        wk_c = np.concatenate([wk[h] for h in hs], axis=1)
        wv_c = np.concatenate([wv[h] for h in hs], axis=1)
        # heads land per pair as [odd head | even head] (B at partitions
        # 0:64, A at 64:128), so wo rows follow that order
        ho = []
        for p in range(NP):
            ho += [hs[2 * p + 1], hs[2 * p]]
        wo_c = np.concatenate([wo[h * DH:(h + 1) * DH, :] for h in ho],
                              axis=0)                               # [512, D]
        bq_c = np.stack([np.concatenate([bq[hs[2 * p]] * scale, bq[hs[2 * p + 1]] * scale])
                         for p in range(NP)], axis=1)               # [128, NP]
        bk_c = np.stack([np.concatenate([bk[hs[2 * p]], bk[hs[2 * p + 1]]])
                         for p in range(NP)], axis=1)
        bv_c = np.concatenate([bv[h] for h in ho])                  # [512]
        bo_eff = bv_c.astype(np.float64) @ wo_c.astype(np.float64)
        if g == 0:
            bo_eff = bo_eff + bo.astype(np.float64)
        bo_c = np.ascontiguousarray(
            bo_eff.astype(np.float32).reshape(8, 128).T)            # [128, 8]
        in_maps.append({
            "query_t": q_t[b],
            "value_t": v_t[b],
            "wq": np.ascontiguousarray(wq_c).astype(bf16),
            "wk": np.ascontiguousarray(wk_c).astype(bf16),
            "wv": np.ascontiguousarray(wv_c).astype(bf16),
            "wo": np.ascontiguousarray(wo_c).astype(bf16),
            "bq": np.ascontiguousarray(bq_c).astype(np.float32),
            "bk": np.ascontiguousarray(bk_c).astype(np.float32),
            "bo": bo_c.astype(np.float32),
        })
    return in_maps


def assemble_output(results, b_cnt=B, s_sz=S):
    out = np.empty((b_cnt, s_sz, D), dtype=np.float32)
    for b in range(b_cnt):
        acc = results[2 * b]["out_t"] + results[2 * b + 1]["out_t"]  # [D, S]
        out[b] = acc.T
    return out


_BUILT = {}


def _get_nc(s_sz=S, repeat=1, phase="full", sc_bufs=2, share_work=False):
    key = (s_sz, repeat, phase, sc_bufs, share_work)
    if key not in _BUILT:
        from concourse import bacc
        nc = bacc.Bacc("TRN2", target_bir_lowering=False, debug=False,
                       num_devices=N_CORES)
        _BUILT[key] = build_kernel(nc, s_sz, repeat, phase, sc_bufs, share_work)
    return _BUILT[key]


def kernel(**inputs):
    from concourse.bass_utils import run_bass_kernel_spmd

    np_inputs = {k: np.asarray(v) for k, v in inputs.items()}
    nc = _get_nc()
    in_maps = make_in_maps(**np_inputs)
    res = run_bass_kernel_spmd(nc, in_maps, list(range(N_CORES)), trace=False)
    return assemble_output(res.results)



# revision 26
# speedup vs baseline: 1.0273x; 1.0273x over previous
"""Trainium2 Bass kernel for nn_MultiHeadAttention_16346645529223.

Full inputs in / full output out. Sharding: (batch x head-half) over the 8
cores -- core c handles batch b = c//2 and heads g*8..g*8+7 with g = c%2.
No device collectives: each core emits the partial output projection
partial_g = heads_g @ wo[g-rows] and the host adds the two partials per
batch (untimed assembly, like the baseline's concat).

Per-core pipeline (b fixed, 8 heads = 4 head-pairs, S = 2048):
  - qT/kT projections with weight-stationary matmuls -> [128 (2 heads x 64
    dims), S] bf16 tiles per pair; softmax scale folded into wq/bq.
  - v projected directly in [kv, dims] orientation (value chunk stationary,
    wv moving) -- no PE transposes. Stored ones-augmented per (pair, kv
    chunk): [64 A-dims | 1 | gap | 1 | zeros | 64 B-dims] so the AV matmul
    also produces the softmax denominator (A at out partition 64, B at out
    partition 0 with its dims at partitions 64..127).
  - scores computed transposed S^T[kv, q] with the two heads of a pair in
    the two 64-row halves of the PE array (concurrent row-group matmuls),
    both heads' chunks in one 2-bank PSUM tile.
  - exp via one ScalarE activation per [128, 1024] PSUM tile -> bf16 xs.
  - AV accumulated over 16 kv chunks; normalization = DVE reciprocal +
    gpsimd partition-broadcast + DVE multiply into bf16 heads tiles.
  - output projection from SBUF heads, + (bo + bv @ wo) bias on g==0 cores
    (zeros on g==1), f32 [1024, S] partial out.

A `repeat` knob wraps the whole per-call body in a tc.For_i loop (consts/
weights stay loaded) so test.py can measure pure device time as the slope
(wall(R) - wall(1)) / (R - 1), cancelling the ~70-90 ms axon dispatch
overhead that dominated the old wall-clock measurement.
"""

import sys

sys.path.insert(0, "/opt/trn_rl_repo")

import numpy as np

N_CORES = 8
B, S, D, H, DH = 4, 2048, 1024, 16, 64
HG = H // 2   # heads per core
NP = HG // 2  # head pairs per core
NDC = D // 128

# vh column layout per (pair, kv-chunk): A dims 0:64, A ones 64, B ones 72,
# B zero block 73:136, B dims 136:200.
VHW = 200


def build_kernel(nc, s_sz=S, repeat=1, phase="full", sc_bufs=2, share_work=False):
    import concourse.tile as tile
    import concourse.mybir as mybir
    from contextlib import ExitStack

    f32 = mybir.dt.float32
    bf16 = mybir.dt.bfloat16
    EXP = mybir.ActivationFunctionType.Exp

    S_ = s_sz
    NQT = S_ // 512
    NKV = S_ // 512
    NKC = S_ // 128

    query_t = nc.dram_tensor("query_t", [D, S_], bf16, kind="ExternalInput")
    value_t = nc.dram_tensor("value_t", [D, S_], bf16, kind="ExternalInput")
    wq_d = nc.dram_tensor("wq", [D, 512], bf16, kind="ExternalInput")
    wk_d = nc.dram_tensor("wk", [D, 512], bf16, kind="ExternalInput")
    wv_d = nc.dram_tensor("wv", [D, 512], bf16, kind="ExternalInput")
    wo_d = nc.dram_tensor("wo", [512, D], bf16, kind="ExternalInput")
    bq_d = nc.dram_tensor("bq", [128, NP], f32, kind="ExternalInput")
    bk_d = nc.dram_tensor("bk", [128, NP], f32, kind="ExternalInput")
    bo_d = nc.dram_tensor("bo", [128, 8], f32, kind="ExternalInput")
    out_d = nc.dram_tensor("out_t", [D, S_], f32, kind="ExternalOutput")

    with tile.TileContext(nc) as tc, ExitStack() as ctx:
        consts = ctx.enter_context(tc.tile_pool(name="consts", bufs=1))
        persist = ctx.enter_context(tc.tile_pool(name="persist", bufs=1))
        stage = ctx.enter_context(tc.tile_pool(name="stage", bufs=2))
        xsp = ctx.enter_context(tc.tile_pool(name="xsp", bufs=10))
        small = ctx.enter_context(tc.tile_pool(name="small", bufs=2))
        pp = ctx.enter_context(tc.tile_pool(name="pp", bufs=2, space="PSUM"))
        scp = ctx.enter_context(tc.tile_pool(name="scp", bufs=sc_bufs, space="PSUM"))
        if share_work:
            avp = pp
        else:
            avp = ctx.enter_context(tc.tile_pool(name="avp", bufs=2, space="PSUM"))

        wq_sb = consts.tile([128, NDC, 512], bf16)
        nc.sync.dma_start(out=wq_sb[:], in_=wq_d[:].rearrange("(dc p) m -> p dc m", p=128))
        wk_sb = consts.tile([128, NDC, 512], bf16)
        nc.sync.dma_start(out=wk_sb[:], in_=wk_d[:].rearrange("(dc p) m -> p dc m", p=128))
        wv_sb = consts.tile([128, NDC, 512], bf16)
        nc.sync.dma_start(out=wv_sb[:], in_=wv_d[:].rearrange("(dc p) m -> p dc m", p=128))
        wo_sb = consts.tile([128, NP, D], bf16)
        nc.sync.dma_start(out=wo_sb[:], in_=wo_d[:].rearrange("(dc p) m -> p dc m", p=128))
        bq_sb = consts.tile([128, NP], f32)
        nc.sync.dma_start(out=bq_sb[:], in_=bq_d[:])
        bk_sb = consts.tile([128, NP], f32)
        nc.sync.dma_start(out=bk_sb[:], in_=bk_d[:])
        bo_sb = consts.tile([128, 8], f32)
        nc.sync.dma_start(out=bo_sb[:], in_=bo_d[:])


        qT = persist.tile([128, NP, S_], bf16)
        kT = persist.tile([128, NP, S_], bf16)
        vh = persist.tile([128, NP, NKC, VHW], bf16)
        heads = persist.tile([128, NP, S_], bf16)

        # static parts of vh (ones columns; zero gap for the B stationary)
        nc.vector.memset(vh[:], 0.0)
        nc.vector.memset(vh[:, :, :, 64:65], 1.0)
        nc.vector.memset(vh[:, :, :, 72:73], 1.0)

        def qproj_closures(qt, tag_sfx=""):
            """Split q projection into small closures for filler pacing."""
            qsl = slice(qt * 512, (qt + 1) * 512)
            state = {}

            def dma():
                qst = stage.tile([128, NDC, 512], bf16, tag="qst",
                                 name=f"qst_{qt}{tag_sfx}")
                nc.sync.dma_start(out=qst[:], in_=query_t[:, qsl].rearrange(
                    "(dc p) m -> p dc m", p=128))
                state["qst"] = qst

            cls = [dma]
            for p in range(NP):
                for dch in range(NDC // 2):
                    def mm2(p=p, dch=dch, first=dch == 0):
                        if first:
                            alloc_p = pp.tile([128, 512], f32, tag="pp",
                                              name=f"psq_{p}_{qt}{tag_sfx}")
                            state["ps"] = alloc_p
                        ps = state["ps"]
                        for dc in (2 * dch, 2 * dch + 1):
                            nc.tensor.matmul(ps[:], wq_sb[:, dc, p * 128:(p + 1) * 128],
                                             state["qst"][:, dc, :],
                                             start=dc == 0, stop=dc == NDC - 1)
                    cls.append(mm2)

                def add(p=p):
                    nc.vector.tensor_scalar_add(qT[:, p, qsl], state["ps"][:],
                                                bq_sb[:, p:p + 1])
                cls.append(add)
            return cls

        def qproj(qt, tag_sfx=""):
            for c in qproj_closures(qt, tag_sfx):
                c()

        def body():
            qproj(0)
            vsts = []
            for kvt in range(NKV):
                ksl = slice(kvt * 512, (kvt + 1) * 512)
                vst = stage.tile([128, NDC, 512], bf16, tag="vst", bufs=NKV,
                                 name=f"vst_{kvt}")
                nc.sync.dma_start(out=vst[:], in_=value_t[:, ksl].rearrange("(dc p) m -> p dc m", p=128))
                vsts.append(vst)

            def kTproj_closures(p):
                cls = []
                state = {}
                for kvt in range(NKV):
                    ksl = slice(kvt * 512, (kvt + 1) * 512)
                    for dch in range(NDC // 2):
                        def mm2(p=p, kvt=kvt, dch=dch, first=dch == 0):
                            if first:
                                state["ps"] = pp.tile([128, 512], f32, tag="pp",
                                                      name=f"psk_{p}_{kvt}")
                            ps = state["ps"]
                            for dc in (2 * dch, 2 * dch + 1):
                                nc.tensor.matmul(ps[:], wk_sb[:, dc, p * 128:(p + 1) * 128],
                                                 vsts[kvt][:, dc, :],
                                                 start=dc == 0, stop=dc == NDC - 1)
                        cls.append(mm2)

                    def add(p=p, kvt=kvt, ksl=ksl):
                        nc.vector.tensor_scalar_add(kT[:, p, ksl], state["ps"][:],
                                                    bk_sb[:, p:p + 1])
                    cls.append(add)
                return cls

            def kTproj(p, kvt):
                cls = kTproj_closures(p)
                for c in cls[kvt * 5:(kvt + 1) * 5]:
                    c()

            def vproj(kc):
                kvt, kcr = divmod(kc, 4)
                ps = pp.tile([128, 512], f32, tag="pp", name=f"psv_{kc}")
                for dc in range(NDC):
                    nc.tensor.matmul(ps[:], vsts[kvt][:, dc, kcr * 128:(kcr + 1) * 128],
                                     wv_sb[:, dc, :], start=dc == 0, stop=dc == NDC - 1)
                pr = ps[:].rearrange("p (g t e) -> p g t e", t=2, e=64)
                nc.vector.tensor_copy(vh[:, :, kc, 0:64], pr[:, :, 0, :])
                nc.vector.tensor_copy(vh[:, :, kc, 136:VHW], pr[:, :, 1, :])

            # kT for pair 0 up front; everything else (v-proj, kT of pairs
            # 1..3) is interleaved into the first q-tile's attention stream
            # below so ACT starts exping almost immediately.
            for kvt in range(NKV):
                kTproj(0, kvt)

            if phase == "proj":
                nc.sync.dma_start(out=out_d[0:128, 0:S_ // 2], in_=qT[:, 0, :].bitcast(f32))
                nc.sync.dma_start(out=out_d[128:256, 0:S_ // 2], in_=kT[:, 0, :].bitcast(f32))
                nc.sync.dma_start(out=out_d[256:384, 0:VHW * NKC // 2], in_=vh[:, 0, :, :].bitcast(f32))
                return

            def oproj_closures(qt):
                """Per output-chunk: 2 matmul closures + 1 add/dma closure."""
                qsl = slice(qt * 512, (qt + 1) * 512)
                state = {}
                cls = []
                for ec in range(8):
                    for ph in range(2):
                        def mm2(ec=ec, ph=ph):
                            if ph == 0:
                                state["ps"] = pp.tile([128, 512], f32, tag="pp",
                                                      name=f"pso_{qt}_{ec}")
                            ps = state["ps"]
                            for p in (2 * ph, 2 * ph + 1):
                                nc.tensor.matmul(ps[:], wo_sb[:, p, ec * 128:(ec + 1) * 128],
                                                 heads[:, p, qsl],
                                                 start=p == 0, stop=p == NP - 1)
                        cls.append(mm2)

                    def add_dma(ec=ec):
                        osb = small.tile([128, 512], f32, tag="osb", name=f"osb_{qt}_{ec}")
                        nc.vector.tensor_scalar_add(osb[:], state["ps"][:], bo_sb[:, ec:ec + 1])
                        nc.sync.dma_start(out=out_d[ec * 128:(ec + 1) * 128, qsl], in_=osb[:])
                    cls.append(add_dma)
                return cls

            def oproj(qt):
                for c in oproj_closures(qt):
                    c()

            pending_muls = []
            for qt in range(NQT):
                qsl = slice(qt * 512, (qt + 1) * 512)
                # lazy fillers: next q-tile's projection + previous q-tile's
                # output projection, paced one small closure at a time into
                # the attention stream so PE never runs a long foreign block
                # while ACT starves or DVE queues back up.
                qp = qproj_closures(qt + 1) if qt + 1 < NQT else []
                op = (oproj_closures(qt - 1)
                      if phase in ("full", "fullnb") and qt > 0 else [])
                # round-robin merge: keeps oproj's DVE bias-adds from
                # clustering right before the unit-end normalize copies
                # (which gate AV PSUM-bank reuse), and starts oproj's
                # heads-consuming matmuls earlier in the q-tile. At most
                # two pp accumulation groups are open at any point, which
                # the 2-buffer pp rotation allows.
                lazy = []
                i = j = 0
                while i < len(qp) or j < len(op):
                    if i < len(qp):
                        lazy.append(qp[i]); i += 1
                    if j < len(op):
                        lazy.append(op[j]); j += 1
                lazy_total = len(lazy)
                lazy_emitted = 0
                SLOTS = NP * NKC

                for p in range(NP):
                    # deferred normalize muls from the previous unit: their
                    # broadcast inputs are ready by now, so they can't
                    # head-of-line-block DVE (which must promptly run the
                    # avsb copies that free the AV PSUM banks).
                    for m in pending_muls:
                        m()
                    pending_muls.clear()
                    do_av = phase not in ("scnx", "scores")
                    if do_av:
                        avA = avp.tile([128, 512], f32, tag="pp" if share_work else "av",
                                       name=f"av_{p}_{qt}_0")
                        avB = avp.tile([128, 512], f32, tag="pp" if share_work else "av",
                                       name=f"av_{p}_{qt}_1")
                    xst = {}

                    def av_mms(kc):
                        # AV matmuls interleaved into the scores stream; the
                        # two heads accumulate in separate banks, scores use
                        # other banks, so the accumulation groups don't mix.
                        nc.tensor.matmul(avA[0:65, :], vh[:, p, kc, 0:65],
                                         xst[kc][:, 0:512],
                                         start=kc == 0, stop=kc == NKC - 1)
                        nc.tensor.matmul(avB[:], vh[:, p, kc, 72:VHW],
                                         xst[kc][:, 512:1024],
                                         start=kc == 0, stop=kc == NKC - 1)

                    # kT for the next pair: urgent fillers that must finish
                    # within this pair's stream (scores of pair p+1 need them)
                    urgent = kTproj_closures(p + 1) if (qt == 0 and p + 1 < NP) else []
                    urgent_emitted = 0

                    LAG = 3
                    for kc in range(NKC):
                        if qt == 0 and p == 0:
                            vproj(kc)
                        kcl = slice(kc * 128, (kc + 1) * 128)
                        sc = scp.tile([128, 1024], f32, tag="sc", name=f"sc_{p}_{qt}_{kc}")
                        nc.tensor.matmul(sc[:, 0:512], kT[0:64, p, kcl], qT[0:64, p, qsl],
                                         start=True, stop=True)
                        nc.tensor.matmul(sc[:, 512:1024], kT[64:128, p, kcl], qT[64:128, p, qsl],
                                         start=True, stop=True)
                        if phase != "scnx":
                            x = xsp.tile([128, 1024], bf16, tag="xs", name=f"xs_{p}_{qt}_{kc}")
                            nc.scalar.activation(x[:], sc[:], EXP)
                            xst[kc] = x
                        if do_av and kc >= LAG:
                            av_mms(kc - LAG)
                        # pacing: keep urgent (kT) on deadline, trickle lazy
                        if urgent:
                            utarget = (len(urgent) * (kc + 1) + NKC - 1) // NKC
                            while urgent_emitted < utarget:
                                urgent[urgent_emitted]()
                                urgent_emitted += 1
                        slot = p * NKC + kc
                        if lazy and slot >= 4:
                            ltarget = min(lazy_total,
                                          lazy_total * (slot - 3) // (SLOTS - 12))
                            while lazy_emitted < ltarget:
                                lazy[lazy_emitted]()
                                lazy_emitted += 1
                    while urgent_emitted < len(urgent):
                        urgent[urgent_emitted]()
                        urgent_emitted += 1
                    if not do_av:
                        continue
                    for kc in range(NKC - LAG, NKC):
                        av_mms(kc)
                    if phase == "av":
                        if p == 0 and qt == 0:
                            avsb = small.tile([128, 512], f32, tag="avsb", name="avsb_dbg")
                            nc.vector.tensor_copy(avsb[:], avB[:])
                            nc.sync.dma_start(out=out_d[128:256, 0:512], in_=avsb[:])
                        continue
                    for h in range(2):
                        av = avA if h == 0 else avB
                        # evacuate PSUM immediately so the next unit's AV can
                        # reuse the bank; the norm chain runs from SBUF.
                        avsb = small.tile([128, 512], f32, tag="avsb", bufs=3,
                                          name=f"avsb_{p}_{qt}_{h}")
                        if h == 0:
                            nc.vector.tensor_copy(avsb[0:65, :], av[0:65, :])
                            rec = small.tile([65, 512], f32, tag="recA", name=f"recA_{p}_{qt}")
                            nc.vector.reciprocal(rec[64:65, :], avsb[64:65, :])
                            bc = small.tile([64, 512], f32, tag="bcA", name=f"bcA_{p}_{qt}")
                            if phase == "fullnb":
                                # timing probe: fake the broadcast with a DVE
                                # copy (wrong values) to isolate Pool/DMA cost
                                nc.vector.tensor_copy(bc[:], avsb[0:64, :])
                            else:
                                # HW partition_broadcast ignores the AP
                                # partition base (broadcasts physical
                                # partition 0), so hop the denominator row
                                # down via a tiny SBUF DMA.
                                st0 = small.tile([1, 512], f32, tag="st0", name=f"st0_{p}_{qt}")
                                nc.sync.dma_start(out=st0[0:1, :], in_=rec[64:65, :])
                                nc.gpsimd.partition_broadcast(bc[:], st0[0:1, :], channels=64)

                            def mulA(avsb=avsb, bc=bc, p=p, qsl=qsl):
                                nc.vector.tensor_mul(heads[0:64, p, qsl], avsb[0:64, :], bc[:])
                            pending_muls.append(mulA)
                        else:
                            nc.vector.tensor_copy(avsb[:], av[:])
                            rec = small.tile([1, 512], f32, tag="recB", name=f"recB_{p}_{qt}")
                            nc.vector.reciprocal(rec[0:1, :], avsb[0:1, :])
                            bc = small.tile([128, 512], f32, tag="bcB", name=f"bcB_{p}_{qt}")
                            if phase == "fullnb":
                                nc.vector.tensor_copy(bc[:], avsb[:])
                            else:
                                nc.gpsimd.partition_broadcast(bc[:], rec[0:1, :], channels=128)

                            def mulB(avsb=avsb, bc=bc, p=p, qsl=qsl):
                                nc.vector.tensor_mul(heads[64:128, p, qsl], avsb[64:128, :], bc[64:128, :])
                            pending_muls.append(mulB)
                    if qt == NQT - 1 and p == NP - 1:
                        # final unit: nothing follows, run the muls now
                        for m in pending_muls:
                            m()
                        pending_muls.clear()
                # drain any lazy fillers not yet emitted (qproj must complete
                # before the next q-tile's scores read qT)
                while lazy_emitted < lazy_total:
                    lazy[lazy_emitted]()
                    lazy_emitted += 1
            if phase in ("full", "fullnb"):
                oproj(NQT - 1)

        if repeat == 1:
            body()
        else:
            # big body (>256 insts/engine) -> arm branch prefetch so the
            # back-edge I$-hits instead of a ~4us IRAM refetch stall
            hints = (mybir.EngineType.PE, mybir.EngineType.Activation,
                     mybir.EngineType.DVE, mybir.EngineType.SP,
                     mybir.EngineType.Pool)
            with tc.For_i(0, repeat, hint_engines=hints):
                body()

    nc.finalize()
    return nc


def make_in_maps(query, value, wq, bq, wk, bk, wv, bv, wo, bo, s_sz=S):
    """Host-side prep: transpose activations, fold scale/bv, shard per core."""
    import ml_dtypes

    bf16 = ml_dtypes.bfloat16
    scale = np.float32(1.0 / np.sqrt(np.float32(DH)))
    b_cnt = query.shape[0]

    q_t = [np.ascontiguousarray(query[b].T).astype(bf16) for b in range(b_cnt)]
    v_t = [np.ascontiguousarray(value[b].T).astype(bf16) for b in range(b_cnt)]

    in_maps = []
    for c in range(N_CORES):
        b, g = c // 2, c % 2
        hs = [g * HG + i for i in range(HG)]
        wq_c = np.concatenate([wq[h] * scale for h in hs], axis=1)  # [D, 512]
        wk_c = np.concatenate([wk[h] for h in hs], axis=1)
        wv_c = np.concatenate([wv[h] for h in hs], axis=1)
        wo_c = wo[hs[0] * DH:(hs[-1] + 1) * DH, :]                  # [512, D]
        bq_c = np.stack([np.concatenate([bq[hs[2 * p]] * scale, bq[hs[2 * p + 1]] * scale])
                         for p in range(NP)], axis=1)               # [128, NP]
        bk_c = np.stack([np.concatenate([bk[hs[2 * p]], bk[hs[2 * p + 1]]])
                         for p in range(NP)], axis=1)
        bv_c = np.concatenate([bv[h] for h in hs])                  # [512]
        bo_eff = bv_c.astype(np.float64) @ wo_c.astype(np.float64)
        if g == 0:
            bo_eff = bo_eff + bo.astype(np.float64)
        bo_c = np.ascontiguousarray(
            bo_eff.astype(np.float32).reshape(8, 128).T)            # [128, 8]
        in_maps.append({
            "query_t": q_t[b],
            "value_t": v_t[b],
            "wq": np.ascontiguousarray(wq_c).astype(bf16),
            "wk": np.ascontiguousarray(wk_c).astype(bf16),
            "wv": np.ascontiguousarray(wv_c).astype(bf16),
            "wo": np.ascontiguousarray(wo_c).astype(bf16),
            "bq": np.ascontiguousarray(bq_c).astype(np.float32),
            "bk": np.ascontiguousarray(bk_c).astype(np.float32),
            "bo": bo_c.astype(np.float32),
        })
    return in_maps


def assemble_output(results, b_cnt=B, s_sz=S):
    out = np.empty((b_cnt, s_sz, D), dtype=np.float32)
    for b in range(b_cnt):
        acc = results[2 * b]["out_t"] + results[2 * b + 1]["out_t"]  # [D, S]
        out[b] = acc.T
    return out


_BUILT = {}


def _get_nc(s_sz=S, repeat=1, phase="full", sc_bufs=2, share_work=False):
    key = (s_sz, repeat, phase, sc_bufs, share_work)
    if key not in _BUILT:
        from concourse import bacc
        nc = bacc.Bacc("TRN2", target_bir_lowering=False, debug=False,
                       num_devices=N_CORES)
        _BUILT[key] = build_kernel(nc, s_sz, repeat, phase, sc_bufs, share_work)
    return _BUILT[key]


def kernel(**inputs):
    from concourse.bass_utils import run_bass_kernel_spmd

    np_inputs = {k: np.asarray(v) for k, v in inputs.items()}
    nc = _get_nc()
    in_maps = make_in_maps(**np_inputs)
    res = run_bass_kernel_spmd(nc, in_maps, list(range(N_CORES)), trace=False)
    return assemble_output(res.results)

